# revision 14
# baseline (speedup 1.0000x reference)
"""MoE layer (N=8192, D=1024, F=4096, E=8, top-2) on 8 Trainium2 NeuronCores.

Strategy (expert-parallel, matches the sharding hint):
  - Host: gate (inputs @ Wg + bg), top-k selection, softmax combine weights,
    dispatch/combine index plumbing, and the FFN for the few overflow tokens
    beyond the per-core device capacity (fp32 numpy; ~300 tokens).
  - Device (SPMD, core e == expert e): the heavy FFN
        y = silu(x_e @ W1[e] + b1[e]) @ W2[e]  scaled per-row by the
    combine weight, for exactly C_DEV = 2048 tokens per core (zero-padded).
    Fixing C_DEV makes all cores perfectly load-balanced; the expert-count
    overflow (cnt_e - 2048, a few hundred tokens total) is done on host.

Per-core kernel layout (all bf16 data, fp32 PSUM accumulate):
  blocks of [768, 768, 512] tokens
  mm1: h^T[f, t] = W1[d, f]^T @ x^T[d, t]   (stationary = W1 tile, moving = x^T)
  silu+bias on ScalarE (PSUM -> SBUF), h^T kept resident in SBUF
  mm2 (tt-major): y[t, d] = h^T[f, t]^T @ W2[f, d] accumulated over f for one
    128-token tile at a time -> drains (vector scale + y DMA) overlap the
    remaining matmuls instead of piling up at the end of each block.
  DMA rings: x + y on sync (HWDGE), w1 on scalar (HWDGE, own ring so the
    first x/w1 transfers run in parallel at startup), w2/b1/cw on gpsimd
    (SWDGE, not needed until later).
"""

import os
import sys
import types

import numpy as np

import concourse.bass as bass
import concourse.bacc as bacc
import concourse.mybir as mybir
import concourse.tile as tile
from concourse.bass_utils import run_bass_kernel_spmd


def _ensure_ntff_hook():
    """Provide antenv.axon_hooks if the image lacks it, so trace=True (or a
    caller-set BASS_TRACE=1) degrades gracefully instead of crashing in
    run_bass_kernel_spmd."""
    try:
        import antenv.axon_hooks  # noqa: F401

        return
    except ImportError:
        pass
    hook = None
    try:
        from trn_agent_boot.trn_boot import _ntff_profile_via_ctypes

        hook = _ntff_profile_via_ctypes("/opt/axon/libaxon_pjrt.so")
    except Exception:
        hook = None
    m = types.ModuleType("antenv.axon_hooks")
    m.get_axon_ntff_profile_hook = lambda: hook
    m.set_axon_ntff_profile_hook = lambda h: None
    sys.modules["antenv.axon_hooks"] = m
    try:
        import antenv

        antenv.axon_hooks = m
    except ImportError:
        pass


_ensure_ntff_hook()

F32 = mybir.dt.float32
BF16 = mybir.dt.bfloat16

D_MODEL = 1024
D_FF = 4096
N_EXPERTS = 8
N_CORES = 8
C_DEV = 2048  # device tokens per core (fixed -> perfectly balanced SPMD)
BLOCKS = [768, 768, 512]
MAX_BLK = 768

# exec time (ns) of the most recent device run, when tracing was enabled
LAST_EXEC_TIME_NS = None
_NC_CACHE = {}


def _split_subtiles(blk):
    out = []
    t = blk
    while t > 0:
        s = min(t, 512)
        out.append(s)
        t -= s
    return out


def _build_nc():
    """bf16 weights fully resident in SBUF; bf16 activations; f32 psum.

    Host pre-shuffles all inputs to partition-major chunk layouts so every
    DMA is 128 fully-contiguous descriptors:
      w1: [8, 128, 8, 512]   (f-chunk, partition, d-chunk, f-within)
      w2: [4, 128, 8, 1024]  (f-chunk, partition, f-within, d)
      x:  [nb, 128, 8, 768]  (block, partition, d-chunk, token)
      b1: [128, 32]  cw: [128, C_DEV/128]
    """
    nc = bacc.Bacc("TRN2", target_bir_lowering=False, debug=False)
    D, F = D_MODEL, D_FF
    nf = F // 128  # 32
    nd = D // 128  # 8
    nb = len(BLOCKS)
    ng = C_DEV // 128

    w1 = nc.declare_dram_parameter("w1", [8, 128, nd, F // 8], BF16, isOutput=False)
    w2 = nc.declare_dram_parameter("w2", [4, 128, nf // 4, D], BF16, isOutput=False)
    xT = nc.declare_dram_parameter("xT", [nb, 128, nd, MAX_BLK], BF16, isOutput=False)
    b1 = nc.declare_dram_parameter("b1", [128, nf], F32, isOutput=False)
    cw = nc.declare_dram_parameter("cw", [128, ng], F32, isOutput=False)
    y = nc.declare_dram_parameter("y", [C_DEV, D], F32, isOutput=True)

    with tile.TileContext(nc) as tc:
        with (
            tc.tile_pool(name="const", bufs=1) as constp,
            tc.tile_pool(name="wres", bufs=1) as wres,
            tc.tile_pool(name="xp", bufs=1) as xp,
            tc.tile_pool(name="hp", bufs=1) as hp,
            tc.tile_pool(name="yp", bufs=3) as yp,
            tc.tile_pool(name="ps1", bufs=4, space="PSUM") as ps1,
            tc.tile_pool(name="ps2", bufs=4, space="PSUM") as ps2,
        ):
            w1_sb = wres.tile([128, 8, nd, F // 8], BF16, tag="w1")
            w2_sb = wres.tile([128, 4, nf // 4, D], BF16, tag="w2")
            x_first = xp.tile([128, nd, MAX_BLK], BF16, tag="x")
            # Startup critical path: x block0 (sync ring, one DMA with fat
            # 12KB/partition descriptors) and w1 chunk0 (scalar ring) transfer
            # concurrently on separate HWDGE rings.  w2 rides the scalar ring
            # FIFO *behind* all of w1, so it cannot steal early bandwidth from
            # the critical transfers (SDMA engines round-robin between queues
            # at packet granularity, so an eager fat queue starves others).
            # Startup critical path: x block0 (sync ring) and w1 chunk0
            # (scalar ring) transfer concurrently on separate HWDGE rings;
            # w2 rides the scalar ring FIFO behind all of w1 so it cannot
            # steal early bandwidth (SDMA engines round-robin between queues
            # at packet granularity).
            nc.sync.dma_start(x_first[:], xT[0])
            for c in range(8):
                nc.scalar.dma_start(w1_sb[:, c], w1[c])
            for c in range(4):
                nc.scalar.dma_start(w2_sb[:, c], w2[c])
            b1_sb = constp.tile([128, nf], F32, tag="b1")
            nc.gpsimd.dma_start(b1_sb[:], b1[:])
            cw_sb = constp.tile([128, ng], F32, tag="cw")
            nc.gpsimd.dma_start(cw_sb[:], cw[:])

            x_tiles = [x_first]
            t0 = 0
            for bi, blk in enumerate(BLOCKS):
                ntt = blk // 128
                x_sb = x_tiles[bi]
                h_sb = hp.tile([128, nf, MAX_BLK], BF16, tag="h")

                # ---- phase 1: h^T = silu(W1^T x^T + b1) ----
                for f in range(nf):
                    s0 = 0
                    for ts in _split_subtiles(blk):
                        ph = ps1.tile([128, 512], F32, tag="ph")
                        for d in range(nd):
                            nc.tensor.matmul(
                                ph[:, :ts],
                                w1_sb[:, f // 4, d, (f % 4) * 128 : (f % 4 + 1) * 128],
                                x_sb[:, d, s0 : s0 + ts],
                                start=(d == 0),
                                stop=(d == nd - 1),
                            )
                        nc.scalar.activation(
                            h_sb[:, f, s0 : s0 + ts],
                            ph[:, :ts],
                            mybir.ActivationFunctionType.Silu,
                            bias=b1_sb[:, f : f + 1],
                        )
                        s0 += ts

                # Prefetch next block's x now: its trigger sits ahead of this
                # block's y DMA triggers on the sync ring, so the transfer
                # overlaps mm2 below instead of waiting for its last drain.
                if bi + 1 < len(BLOCKS):
                    x_next = xp.tile([128, nd, MAX_BLK], BF16, tag="x")
                    nc.sync.dma_start(x_next[:], xT[bi + 1])
                    x_tiles.append(x_next)

                # ---- phase 2 (tt-major): y tile = (h^T)^T W2, scaled ----
                for tt in range(ntt):
                    pyA = ps2.tile([128, 512], F32, tag="py", name="pyA")
                    pyB = ps2.tile([128, 512], F32, tag="py", name="pyB")
                    for f in range(nf):
                        h_st = h_sb[:, f, tt * 128 : (tt + 1) * 128]
                        nc.tensor.matmul(
                            pyA[:],
                            h_st,
                            w2_sb[:, f // 8, f % 8, 0:512],
                            start=(f == 0),
                            stop=(f == nf - 1),
                        )
                        nc.tensor.matmul(
                            pyB[:],
                            h_st,
                            w2_sb[:, f // 8, f % 8, 512:1024],
                            start=(f == 0),
                            stop=(f == nf - 1),
                        )
                    g = t0 // 128 + tt
                    y_sb = yp.tile([128, D], F32, tag="y")
                    nc.vector.tensor_scalar_mul(
                        y_sb[:, 0:512], pyA[:], cw_sb[:, g : g + 1]
                    )
                    nc.vector.tensor_scalar_mul(
                        y_sb[:, 512:1024], pyB[:], cw_sb[:, g : g + 1]
                    )
                    nc.sync.dma_start(
                        y[t0 + tt * 128 : t0 + (tt + 1) * 128, :], y_sb[:]
                    )
                t0 += blk
    nc.finalize()  # Bacc: runs wait-legalization + register allocation
    return nc


def _route(inputs, Wg, bg, k):
    """Host gate: replicate reference numerics (fp32) for routing."""
    logits = inputs.astype(np.float32) @ Wg.astype(np.float32) + bg.astype(np.float32)
    sel = np.argsort(-logits, axis=1, kind="stable")[:, :k]  # == jax.lax.top_k order
    tl = np.take_along_axis(logits, sel, axis=1).astype(np.float32)
    m = tl.max(axis=1, keepdims=True)
    e = np.exp(tl - m, dtype=np.float32)
    w = (e / e.sum(axis=1, keepdims=True)).astype(np.float32)
    return sel, w


def kernel(inputs, Wg, bg, W1, b1, W2, b2, k):
    global LAST_EXEC_TIME_NS
    import ml_dtypes

    k = int(np.asarray(k))
    inputs = np.ascontiguousarray(np.asarray(inputs, dtype=np.float32))
    Wg = np.asarray(Wg, dtype=np.float32)
    bg = np.asarray(bg, dtype=np.float32)
    W1 = np.asarray(W1, dtype=np.float32)
    b1 = np.asarray(b1, dtype=np.float32)
    W2 = np.asarray(W2, dtype=np.float32)
    b2 = np.asarray(b2, dtype=np.float32)

    N, D = inputs.shape
    E = Wg.shape[1]
    assert E == N_EXPERTS and D == D_MODEL and W1.shape == (E, D, D_FF)

    sel, w = _route(inputs, Wg, bg, k)

    # per-expert token lists; device takes the first C_DEV, host the rest
    idxs, wvals = [], []
    for e in range(E):
        tok, slot = np.nonzero(sel == e)
        idxs.append(tok)
        wvals.append(w[tok, slot])

    wdt = ml_dtypes.bfloat16
    nb = len(BLOCKS)
    Cp = nb * MAX_BLK  # x padded to whole (possibly oversized) blocks
    in_maps = []
    for e in range(E):
        cnt = min(len(idxs[e]), C_DEV)
        cwe = np.zeros((C_DEV,), dtype=np.float32)
        cwe[:cnt] = wvals[e][:cnt]
        xe = np.zeros((Cp, D), dtype=wdt)
        xe[:cnt] = inputs[idxs[e][:cnt]].astype(wdt)
        # [Cp, D] -> [nb, 128, 8, MAX_BLK]: t=(b, t'), d=(a, p)
        xe = np.ascontiguousarray(
            xe.reshape(nb, MAX_BLK, 8, 128).transpose(0, 3, 2, 1)
        )
        w1e = np.ascontiguousarray(
            W1[e].astype(wdt).reshape(8, 128, 8, 512).transpose(2, 1, 0, 3)
        )  # [fc, p, d-chunk, f-within]
        w2e = np.ascontiguousarray(
            W2[e].astype(wdt).reshape(4, 8, 128, D).transpose(0, 2, 1, 3)
        )  # [fc, p, f-within, d]
        b1e = np.ascontiguousarray(b1[e].reshape(32, 128).T)
        cwe = np.ascontiguousarray(cwe.reshape(C_DEV // 128, 128).T)
        in_maps.append({"xT": xe, "w1": w1e, "b1": b1e, "w2": w2e, "cw": cwe})

    if "nc" not in _NC_CACHE:
        _NC_CACHE["nc"] = _build_nc()
    nc = _NC_CACHE["nc"]

    trace = bool(os.environ.get("BASS_TRACE"))
    res = None
    for attempt in range(3):
        try:
            res = run_bass_kernel_spmd(
                nc, in_maps, core_ids=list(range(N_CORES)), trace=trace
            )
            break
        except Exception:
            # transient NRT/device failures recover after a short pause
            if attempt == 2:
                raise
            import time

            time.sleep(20)
    LAST_EXEC_TIME_NS = getattr(res, "exec_time_ns", None)

    results = np.zeros((N, D), dtype=np.float32)
    for e in range(E):
        cnt = min(len(idxs[e]), C_DEV)
        ye = np.asarray(res.results[e]["y"])[:cnt]
        # device computed w * (silu(x W1 + b1) @ W2); add the w * b2[e] term
        results[idxs[e][:cnt]] += ye + wvals[e][:cnt, None] * b2[e][None, :]
        if len(idxs[e]) > cnt:  # overflow tokens: fp32 FFN on host
            tok_o = idxs[e][cnt:]
            w_o = wvals[e][cnt:]
            xo = inputs[tok_o]
            h = xo @ W1[e] + b1[e]
            h = h * (1.0 / (1.0 + np.exp(-h)))
            yo = h @ W2[e] + b2[e]
            results[tok_o] += w_o[:, None] * yo
    return results.astype(np.float32)
